# revision 25
# baseline (speedup 1.0000x reference)
"""Trainium2 Bass kernel for the HOI relation model.

Per core (2 images, 8 cores data-parallel over batch):
  1. ROI mean pooling as K-chunk matmuls over a *packed* pixel list:
     only pixels inside the union of the 32 boxes are shipped/streamed
     (~44% of the 64x64 grid).  Features and masks travel as fp8 e3m4
     (4 mantissa bits), halving DMA bytes vs bf16; PSUM accumulates f32.
  2. PE-transpose pooled [32,768] -> pooledT [768, img, det] bf16.
  3. Layer 1 factorized: relu(pair(h,o) @ w1 + b1) = relu(A(h)+B(o)+b1),
     A = w1[:768].T @ h, B = w1[768:].T @ o; the 8x24 pair expansion
     happens after the matmul, split across DVE (add) + ACT (bias+relu).
  4. Layers 2, 3 as plain matmuls on the 384 pair rows.

Scheduling tricks:
  - dummy matmuls bridge the DMA prefix so the PE HAM clock-gate is warm
    (2.4 GHz) when pooling starts;
  - MLP weights ride a single bf16 blob whose DMA is released only after
    the last feature DMA lands (WAW hazard on a 1-element copy), so the
    feature stream gets exclusive bandwidth first;
  - w1 is mc-major so layer 1 starts after half the blob has arrived.

All DRAM tensors are laid out partition-major on the host so every DMA
moves large contiguous per-partition lines. Host does layout/packing
prep only: box rasterization, score argsort (baked into mask column
order), union-pixel gather, dtype casts.
"""

import numpy as np
import ml_dtypes

import concourse.bass as bass
import concourse.mybir as mybir
import concourse.tile as tile
from concourse import bacc
from concourse.bass_utils import run_bass_kernel_spmd
from concourse.masks import make_identity

N_CORES = 8
B, D, C = 16, 32, 768
NH, NO = 8, 24
NPAIR = NH * NO              # 192 pairs per image
GRID = 64                    # feature grid (896 / 14)
BL = B // N_CORES            # 2 images per core
H1, H2, H3 = 512, 256, 117
M = BL * NPAIR               # 384 pair rows per core
NPRE = 4                     # PE prewarm dummy matmuls

# weights blob column offsets (bf16, per partition)
W2_OFF, W3_OFF = 0, 4 * H2
B1_OFF = W3_OFF + 2 * H3
B2_OFF = B1_OFF + 4
B3_OFF = B2_OFF + 2
WC_COLS = B3_OFF + H3

F32 = mybir.dt.float32
BF16 = mybir.dt.bfloat16
FP8 = mybir.dt.float8e3
BF = ml_dtypes.bfloat16
F8 = ml_dtypes.float8_e3m4

_PROGRAMS = {}


def _build_program(nch):
    """nch: number of 128-pixel K-chunks per image (padded packed pixels)."""
    nc = bacc.Bacc("TRN2", target_bir_lowering=False, debug=False,
                   num_devices=N_CORES)
    feat = nc.declare_dram_parameter("feat", [128, BL, nch, C], FP8,
                                     isOutput=False)
    maskT = nc.declare_dram_parameter("maskT", [128, BL, nch, D], FP8,
                                      isOutput=False)
    inva = nc.declare_dram_parameter("inva", [D, BL], F32, isOutput=False)
    w1 = nc.declare_dram_parameter("w1", [128, 4, 12, 128], BF16,
                                   isOutput=False)
    wc = nc.declare_dram_parameter("wc", [128, WC_COLS], BF16, isOutput=False)
    out = nc.declare_dram_parameter("out", [128, 3, H3], F32, isOutput=True)

    add = mybir.AluOpType.add
    relu = mybir.ActivationFunctionType.Relu

    # feature DMA piece sizes: first piece small so compute starts early,
    # later pieces big enough that the ~0.65us-per-issue rate keeps the
    # queue fed; all on ONE queue so pieces complete in issue order
    def _grow(n, sizes):
        out, left = [], n
        for s in sizes:
            if left <= 0:
                break
            out.append(min(s, left))
            left -= out[-1]
        if left > 0:
            out.append(left)
        return out
    pieces0 = _grow(nch, [2, 4, 4])
    pieces1 = _grow(nch, [5, 5])

    with tile.TileContext(nc) as tc:
        with (
            tc.tile_pool(name="singles", bufs=1) as singles,
            tc.tile_pool(name="work", bufs=1) as work,
            tc.tile_pool(name="tmp", bufs=3) as tmpp,
            tc.tile_pool(name="pps", bufs=1, space="PSUM") as pps,
            tc.tile_pool(name="mps", bufs=4, space="PSUM") as mps,
        ):
            ident = singles.tile([32, 32], BF16, tag="ident")
            make_identity(nc, ident)
            # ---- PE prewarm: bridge the DMA prefix (alternating banks) ----
            scr = singles.tile([32, 384], BF16, tag="scr")
            nc.vector.memset(scr, 0.0)
            ps_a0 = pps.tile([D, 384], F32, tag="pp0a")
            ps_b0 = pps.tile([D, 384], F32, tag="pp0b")
            for i in range(NPRE):
                nc.tensor.matmul(ps_a0 if i % 2 == 0 else ps_b0,
                                 scr[:, 0:32], scr, start=True, stop=True)

            # ---- DMA: features exclusively on the Sync queue (in order);
            #      mask/inva/weights on the Scalar queue ----
            inva_sb = singles.tile([D, BL], F32, tag="inva")
            nc.scalar.dma_start(out=inva_sb, in_=inva[:, :])
            m_sb = singles.tile([128, BL, nch, D], FP8, tag="mask")
            nc.scalar.dma_start(out=m_sb, in_=maskT[:, :, :, :])
            f_sb = []
            for img in range(BL):
                pieces = pieces0 if img == 0 else pieces1
                tiles, base = [], 0
                for pi, p in enumerate(pieces):
                    ft = singles.tile([128, p, C], FP8, tag=f"feat{img}_{pi}")
                    nc.sync.dma_start(out=ft, in_=feat[:, img, base:base + p, :])
                    tiles.append((base, ft))
                    base += p
                f_sb.append(tiles)

            # weights ride the SAME Sync queue behind the features: the queue
            # is FIFO, so they start streaming exactly when features finish —
            # no triggers, no cross-queue bandwidth contention.  w1 is split
            # per-mc so layer 1 starts as each quarter lands.
            w1_sb = singles.tile([128, 4, 12, 128], BF16, tag="w1")
            wc_sb = singles.tile([128, WC_COLS], BF16, tag="wc")
            for mc in range(4):
                nc.sync.dma_start(out=w1_sb[:, mc, :, :], in_=w1[:, mc, :, :])
            nc.sync.dma_start(out=wc_sb, in_=wc[:, :])
            w2v = wc_sb[:, W2_OFF:W2_OFF + 4 * H2] \
                .rearrange("p (k n) -> p k n", k=4)
            w3v = wc_sb[:, W3_OFF:W3_OFF + 2 * H3] \
                .rearrange("p (k n) -> p k n", k=2)
            b1v = wc_sb[:, B1_OFF:B1_OFF + 4]
            b2v = wc_sb[:, B2_OFF:B2_OFF + 2]
            b3v = wc_sb[:, B3_OFF:B3_OFF + H3]

            # persistent activations: pooledT [chan, kc, img, det]
            pooledT = work.tile([128, 6, BL, D], BF16, tag="pooledT")
            x1T = work.tile([128, 4, M], BF16, tag="x1T")
            x2T = work.tile([128, 2, M], BF16, tag="x2T")

            # ---- pooling + transpose per image ----
            # all epilogues on DVE: the ACT queue holds the blocked weight-DMA
            # issues during this phase and must not carry pooling-path work
            for img in range(BL):
                ps_a = ps_a0 if img == 0 else pps.tile([D, 384], F32, tag="pp1a")
                ps_b = ps_b0 if img == 0 else pps.tile([D, 384], F32, tag="pp1b")
                for base, ft in f_sb[img]:
                    for kl in range(ft.shape[1]):
                        kk = base + kl
                        nc.tensor.matmul(ps_a, m_sb[:, img, kk, :],
                                         ft[:, kl, 0:384],
                                         start=(kk == 0), stop=(kk == nch - 1))
                        nc.tensor.matmul(ps_b, m_sb[:, img, kk, :],
                                         ft[:, kl, 384:768],
                                         start=(kk == 0), stop=(kk == nch - 1))
                        if img == 1 and kk % 3 == 1:
                            # keep PE busy across DMA-gated stretches (img0's
                            # banks are free again once its epilogue has read)
                            pf = ps_a0 if kk % 2 == 0 else ps_b0
                            nc.tensor.matmul(pf[:, 0:128], scr[:, 0:32],
                                             scr[:, 0:128], start=True, stop=True)
                # scale by 1/area, cast to bf16
                pooled = tmpp.tile([D, C], BF16, tag="pooled")
                nc.vector.tensor_scalar_mul(pooled[:, 0:384], ps_a,
                                            inva_sb[:, img:img + 1])
                nc.vector.tensor_scalar_mul(pooled[:, 384:768], ps_b,
                                            inva_sb[:, img:img + 1])
                # transpose to [C, D] in 6 chunks of 128 channels
                for cc in range(6):
                    ps_t = mps.tile([128, D], BF16, tag="mm")
                    nc.tensor.transpose(ps_t, pooled[:, cc * 128:(cc + 1) * 128],
                                        ident)
                    nc.vector.tensor_copy(pooledT[:, cc, img, :], ps_t)

            # keep the PE HAM clock warm across short dependency stalls
            def fillers(n):
                for i in range(n):
                    nc.tensor.matmul(ps_a0 if i % 2 == 0 else ps_b0,
                                     scr[:, 0:32], scr, start=True, stop=True)

            # ---- layer 1 (factorized over pairs) ----
            fillers(2)
            for mc in range(4):
                ps_h = mps.tile([128, BL, NH], F32, tag="mm")
                ps_o = mps.tile([128, BL, NO], F32, tag="mm")
                for kc in range(6):
                    nc.tensor.matmul(ps_h, w1_sb[:, mc, kc, :],
                                     pooledT[:, kc, :, 0:NH],
                                     start=(kc == 0), stop=(kc == 5))
                for kc in range(6):
                    nc.tensor.matmul(ps_o, w1_sb[:, mc, 6 + kc, :],
                                     pooledT[:, kc, :, NH:D],
                                     start=(kc == 0), stop=(kc == 5))
                a_sb = tmpp.tile([128, BL, NH], BF16, tag="ab")
                nc.scalar.copy(a_sb, ps_h)
                for img in range(BL):
                    pre = tmpp.tile([128, NH, NO], BF16, tag="pre")
                    a_bc = a_sb[:, img, :][:, :, None].broadcast_to([128, NH, NO])
                    b_bc = ps_o[:, img, :][:, None, :].broadcast_to([128, NH, NO])
                    nc.vector.tensor_tensor(pre, a_bc, b_bc, op=add)
                    dst = x1T[:, mc, img * NPAIR:(img + 1) * NPAIR] \
                        .rearrange("p (i j) -> p i j", i=NH)
                    nc.scalar.activation(dst, pre, relu, bias=b1v[:, mc:mc + 1])

            # ---- layer 2 ----
            fillers(4)
            for m2 in range(2):
                ps2 = mps.tile([128, M], F32, tag="mm")
                for kc in range(4):
                    nc.tensor.matmul(ps2, w2v[:, kc, m2 * 128:(m2 + 1) * 128],
                                     x1T[:, kc, :], start=(kc == 0), stop=(kc == 3))
                nc.scalar.activation(x2T[:, m2, :], ps2, relu,
                                     bias=b2v[:, m2:m2 + 1])

            # ---- layer 3 + bias + single store ----
            fillers(3)
            o_sb = tmpp.tile([128, 3, H3], F32, tag="osb")
            for m3 in range(3):
                ps3 = mps.tile([128, H3], F32, tag="mm")
                for kc in range(2):
                    nc.tensor.matmul(ps3, x2T[:, kc, m3 * 128:(m3 + 1) * 128],
                                     w3v[:, kc, :], start=(kc == 0), stop=(kc == 1))
                nc.vector.tensor_tensor(o_sb[:, m3, :], ps3,
                                        b3v[:, 0:H3], op=add)
                nc.scalar.dma_start(out=out[:, m3, :], in_=o_sb[:, m3, :])
    nc.compile()
    return nc


def _get_program(nch):
    if nch not in _PROGRAMS:
        _PROGRAMS[nch] = _build_program(nch)
    return _PROGRAMS[nch]


def _preprocess(features, boxes, scores):
    """Pack union-of-boxes pixels; rasterize masks with detection columns in
    sorted-score order. Returns partition-major fp8 feat/mask + 1/area."""
    cx, cy, bw, bh = boxes[..., 0], boxes[..., 1], boxes[..., 2], boxes[..., 3]
    x1 = np.floor((cx - bw / 2) * GRID).astype(np.int64)
    y1 = np.floor((cy - bh / 2) * GRID).astype(np.int64)
    x2 = np.floor((cx + bw / 2) * GRID).astype(np.int64)
    y2 = np.floor((cy + bh / 2) * GRID).astype(np.int64)
    hidx = np.argsort(-scores[:, :NH], axis=1, kind="stable")
    oidx = np.argsort(-scores[:, NH:], axis=1, kind="stable") + NH
    perm = np.concatenate([hidx, oidx], axis=1)                     # [B, D]
    g = np.arange(GRID)
    rows = (g[None, None, :] >= y1[..., None]) & (g[None, None, :] < y2[..., None])
    cols = (g[None, None, :] >= x1[..., None]) & (g[None, None, :] < x2[..., None])
    rows = np.take_along_axis(rows, perm[..., None], axis=1)        # [B, D, 64]
    cols = np.take_along_axis(cols, perm[..., None], axis=1)
    area = rows.sum(-1) * cols.sum(-1)                              # [B, D]
    # union of all boxes per image; pack only covered pixels
    union = (rows[:, :, :, None] & cols[:, :, None, :]).any(axis=1)  # [B,64,64]
    uflat = union.reshape(B, GRID * GRID)
    counts = uflat.sum(1)
    nch = int(-(-counts.max() // 128))
    pp = nch * 128
    pix = np.zeros((B, pp), np.int64)
    valid = np.zeros((B, pp), bool)
    for b in range(B):
        idx = np.flatnonzero(uflat[b])
        pix[b, :len(idx)] = idx
        valid[b, :len(idx)] = True
    featP = features.reshape(B, GRID * GRID, C)[np.arange(B)[:, None], pix]
    featP[~valid] = 0.0
    py, px = pix // GRID, pix % GRID
    mrow = np.take_along_axis(rows, py[:, None, :], axis=2)          # [B, D, pp]
    mcol = np.take_along_axis(cols, px[:, None, :], axis=2)
    maskP = (mrow & mcol & valid[:, None, :]).transpose(0, 2, 1)     # [B, pp, D]
    return (featP.astype(F8), maskP.astype(F8),
            (1.0 / area).astype(np.float32), nch)


def _pmajor(a, nch):
    """[BL, nch*128, X] -> contiguous [128, BL, nch, X]."""
    bl, _, x = a.shape
    return np.ascontiguousarray(
        a.reshape(bl, nch, 128, x).transpose(2, 0, 1, 3))


def _run(in_maps, trace=False, **kw):
    nch = in_maps[0]["feat"].shape[2]
    nc = _get_program(nch)
    return run_bass_kernel_spmd(nc, in_maps, core_ids=list(range(N_CORES)),
                                trace=trace, **kw)


def _make_in_maps(features, boxes, scores, w1, b1, w2, b2, w3, b3):
    features = np.asarray(features, np.float32)
    featP, maskP, inva, nch = _preprocess(
        features, np.asarray(boxes, np.float32), np.asarray(scores, np.float32))
    # w1 [1536, 512] -> [128 part, 4 mc, 12 kc, 128]
    w1b = np.asarray(w1, np.float32).astype(BF).reshape(12, 128, 4, 128)
    w1b = np.ascontiguousarray(w1b.transpose(1, 2, 0, 3))
    wcb = np.zeros((128, WC_COLS), BF)
    wcb[:, W2_OFF:W2_OFF + 4 * H2] = np.asarray(w2, np.float32).astype(BF) \
        .reshape(4, 128, H2).transpose(1, 0, 2).reshape(128, 4 * H2)
    wcb[:, W3_OFF:W3_OFF + 2 * H3] = np.asarray(w3, np.float32).astype(BF) \
        .reshape(2, 128, H3).transpose(1, 0, 2).reshape(128, 2 * H3)
    wcb[:, B1_OFF:B1_OFF + 4] = np.asarray(b1, np.float32).astype(BF) \
        .reshape(4, 128).T
    wcb[:, B2_OFF:B2_OFF + 2] = np.asarray(b2, np.float32).astype(BF) \
        .reshape(2, 128).T
    wcb[:, B3_OFF:B3_OFF + H3] = np.asarray(b3, np.float32).astype(BF)[None, :]
    wcb = np.ascontiguousarray(wcb)
    in_maps = []
    for c in range(N_CORES):
        s = slice(c * BL, (c + 1) * BL)
        in_maps.append({
            "feat": _pmajor(featP[s], nch),
            "maskT": _pmajor(maskP[s], nch),
            "inva": np.ascontiguousarray(inva[s].T),
            "w1": w1b, "wc": wcb,
        })
    return in_maps


def kernel(features, boxes, scores, w1, b1, w2, b2, w3, b3, labels):
    in_maps = _make_in_maps(features, boxes, scores, w1, b1, w2, b2, w3, b3)
    res = _run(in_maps, trace=False)
    out = np.concatenate(
        [r["out"].transpose(1, 0, 2).reshape(BL, NPAIR, H3)
         for r in res.results], axis=0)
    return np.ascontiguousarray(out.astype(np.float32))


# revision 26
# speedup vs baseline: 1.2330x; 1.2330x over previous
"""Trainium2 Bass kernel for the HOI relation model.

Per core (2 images, 8 cores data-parallel over batch):
  1. ROI mean pooling as K-chunk matmuls over a *packed* pixel list:
     only pixels inside the union of the 32 boxes are shipped/streamed
     (~44% of the 64x64 grid).  Features and masks travel as fp8 e3m4
     (4 mantissa bits), halving DMA bytes vs bf16; PSUM accumulates f32.
  2. PE-transpose pooled [32,768] -> pooledT [768, img, det] bf16.
  3. Layer 1 factorized: relu(pair(h,o) @ w1 + b1) = relu(A(h)+B(o)+b1),
     A = w1[:768].T @ h, B = w1[768:].T @ o; the 8x24 pair expansion
     happens after the matmul, split across DVE (add) + ACT (bias+relu).
  4. Layers 2, 3 as plain matmuls on the 384 pair rows.

Scheduling tricks:
  - dummy matmuls bridge the DMA prefix so the PE HAM clock-gate is warm
    (2.4 GHz) when pooling starts;
  - MLP weights ride a single bf16 blob whose DMA is released only after
    the last feature DMA lands (WAW hazard on a 1-element copy), so the
    feature stream gets exclusive bandwidth first;
  - w1 is mc-major so layer 1 starts after half the blob has arrived.

All DRAM tensors are laid out partition-major on the host so every DMA
moves large contiguous per-partition lines. Host does layout/packing
prep only: box rasterization, score argsort (baked into mask column
order), union-pixel gather, dtype casts.
"""

import numpy as np
import ml_dtypes

import concourse.bass as bass
import concourse.mybir as mybir
import concourse.tile as tile
from concourse import bacc
from concourse.bass_utils import run_bass_kernel_spmd
from concourse.masks import make_identity

N_CORES = 8
B, D, C = 16, 32, 768
NH, NO = 8, 24
NPAIR = NH * NO              # 192 pairs per image
GRID = 64                    # feature grid (896 / 14)
BL = B // N_CORES            # 2 images per core
H1, H2, H3 = 512, 256, 117
M = BL * NPAIR               # 384 pair rows per core
NPRE = 4                     # PE prewarm dummy matmuls

# weights blob column offsets (bf16, per partition)
W2_OFF, W3_OFF = 0, 4 * H2
B1_OFF = W3_OFF + 2 * H3
B2_OFF = B1_OFF + 4
B3_OFF = B2_OFF + 2
WC_COLS = B3_OFF + H3

F32 = mybir.dt.float32
BF16 = mybir.dt.bfloat16
FP8 = mybir.dt.float8e3
BF = ml_dtypes.bfloat16
F8 = ml_dtypes.float8_e3m4

_PROGRAMS = {}


def _build_program(nch):
    """nch: number of 128-pixel K-chunks per image (padded packed pixels)."""
    nc = bacc.Bacc("TRN2", target_bir_lowering=False, debug=False,
                   num_devices=N_CORES)
    feat = nc.declare_dram_parameter("feat", [128, BL, nch, C], FP8,
                                     isOutput=False)
    maskT = nc.declare_dram_parameter("maskT", [128, BL, nch, D], FP8,
                                      isOutput=False)
    inva = nc.declare_dram_parameter("inva", [D, BL], F32, isOutput=False)
    w1 = nc.declare_dram_parameter("w1", [128, 4, 12, 128], BF16,
                                   isOutput=False)
    wc = nc.declare_dram_parameter("wc", [128, WC_COLS], BF16, isOutput=False)
    out = nc.declare_dram_parameter("out", [128, 3, H3], F32, isOutput=True)

    add = mybir.AluOpType.add
    relu = mybir.ActivationFunctionType.Relu

    # feature DMA piece sizes: first piece small so compute starts early,
    # later pieces big enough that the ~0.65us-per-issue rate keeps the
    # queue fed; all on ONE queue so pieces complete in issue order
    def _grow(n, sizes):
        out, left = [], n
        for s in sizes:
            if left <= 0:
                break
            out.append(min(s, left))
            left -= out[-1]
        if left > 0:
            out.append(left)
        return out
    pieces0 = _grow(nch, [2, 4, 4])
    pieces1 = _grow(nch, [5, 5])

    with tile.TileContext(nc) as tc:
        with (
            tc.tile_pool(name="singles", bufs=1) as singles,
            tc.tile_pool(name="work", bufs=1) as work,
            tc.tile_pool(name="tmp", bufs=3) as tmpp,
            tc.tile_pool(name="pps", bufs=1, space="PSUM") as pps,
            tc.tile_pool(name="mps", bufs=4, space="PSUM") as mps,
        ):
            ident = singles.tile([32, 32], BF16, tag="ident")
            make_identity(nc, ident)
            # ---- PE prewarm: bridge the DMA prefix (alternating banks) ----
            scr = singles.tile([32, 384], BF16, tag="scr")
            nc.vector.memset(scr, 0.0)
            ps_a0 = pps.tile([D, 384], F32, tag="pp0a")
            ps_b0 = pps.tile([D, 384], F32, tag="pp0b")
            for i in range(NPRE):
                nc.tensor.matmul(ps_a0 if i % 2 == 0 else ps_b0,
                                 scr[:, 0:32], scr, start=True, stop=True)

            # ---- DMA: features exclusively on the Sync queue (in order);
            #      mask (first! it gates every pooling matmul) + inva on the
            #      Scalar queue ----
            m_sb = singles.tile([128, BL, nch, D], FP8, tag="mask")
            nc.scalar.dma_start(out=m_sb, in_=maskT[:, :, :, :])
            inva_sb = singles.tile([D, BL], F32, tag="inva")
            nc.scalar.dma_start(out=inva_sb, in_=inva[:, :])
            f_sb = []
            for img in range(BL):
                pieces = pieces0 if img == 0 else pieces1
                tiles, base = [], 0
                for pi, p in enumerate(pieces):
                    ft = singles.tile([128, p, C], FP8, tag=f"feat{img}_{pi}")
                    nc.sync.dma_start(out=ft, in_=feat[:, img, base:base + p, :])
                    tiles.append((base, ft))
                    base += p
                f_sb.append(tiles)

            # weights ride the SAME Sync queue behind the features: the queue
            # is FIFO, so they start streaming exactly when features finish —
            # no triggers, no cross-queue bandwidth contention.  w1 is split
            # per-mc so layer 1 starts as each quarter lands.
            w1_sb = singles.tile([128, 4, 12, 128], BF16, tag="w1")
            wc_sb = singles.tile([128, WC_COLS], BF16, tag="wc")
            for mc in range(4):
                nc.sync.dma_start(out=w1_sb[:, mc, :, :], in_=w1[:, mc, :, :])
            nc.sync.dma_start(out=wc_sb, in_=wc[:, :])
            w2v = wc_sb[:, W2_OFF:W2_OFF + 4 * H2] \
                .rearrange("p (k n) -> p k n", k=4)
            w3v = wc_sb[:, W3_OFF:W3_OFF + 2 * H3] \
                .rearrange("p (k n) -> p k n", k=2)
            b1v = wc_sb[:, B1_OFF:B1_OFF + 4]
            b2v = wc_sb[:, B2_OFF:B2_OFF + 2]
            b3v = wc_sb[:, B3_OFF:B3_OFF + H3]

            # persistent activations: pooledT [chan, kc, img, det]
            pooledT = work.tile([128, 6, BL, D], BF16, tag="pooledT")
            x1T = work.tile([128, 4, M], BF16, tag="x1T")
            x2T = work.tile([128, 2, M], BF16, tag="x2T")

            # ---- pooling + transpose per image ----
            # all epilogues on DVE: the ACT queue holds the blocked weight-DMA
            # issues during this phase and must not carry pooling-path work
            for img in range(BL):
                ps_a = ps_a0 if img == 0 else pps.tile([D, 384], F32, tag="pp1a")
                ps_b = ps_b0 if img == 0 else pps.tile([D, 384], F32, tag="pp1b")
                for base, ft in f_sb[img]:
                    for kl in range(ft.shape[1]):
                        kk = base + kl
                        nc.tensor.matmul(ps_a, m_sb[:, img, kk, :],
                                         ft[:, kl, 0:384],
                                         start=(kk == 0), stop=(kk == nch - 1))
                        nc.tensor.matmul(ps_b, m_sb[:, img, kk, :],
                                         ft[:, kl, 384:768],
                                         start=(kk == 0), stop=(kk == nch - 1))
                        if img == 1 and kk % 3 == 1:
                            # keep PE busy across DMA-gated stretches (img0's
                            # banks are free again once its epilogue has read)
                            pf = ps_a0 if kk % 2 == 0 else ps_b0
                            nc.tensor.matmul(pf[:, 0:128], scr[:, 0:32],
                                             scr[:, 0:128], start=True, stop=True)
                # scale by 1/area, cast to bf16
                pooled = tmpp.tile([D, C], BF16, tag="pooled")
                nc.vector.tensor_scalar_mul(pooled[:, 0:384], ps_a,
                                            inva_sb[:, img:img + 1])
                nc.vector.tensor_scalar_mul(pooled[:, 384:768], ps_b,
                                            inva_sb[:, img:img + 1])
                # transpose to [C, D] in 6 chunks of 128 channels
                for cc in range(6):
                    ps_t = mps.tile([128, D], BF16, tag="mm")
                    nc.tensor.transpose(ps_t, pooled[:, cc * 128:(cc + 1) * 128],
                                        ident)
                    nc.vector.tensor_copy(pooledT[:, cc, img, :], ps_t)

            # keep the PE HAM clock warm across short dependency stalls
            def fillers(n):
                for i in range(n):
                    nc.tensor.matmul(ps_a0 if i % 2 == 0 else ps_b0,
                                     scr[:, 0:32], scr, start=True, stop=True)

            # ---- layer 1 (factorized over pairs) ----
            fillers(2)
            for mc in range(4):
                ps_h = mps.tile([128, BL, NH], F32, tag="mm")
                ps_o = mps.tile([128, BL, NO], F32, tag="mm")
                for kc in range(6):
                    nc.tensor.matmul(ps_h, w1_sb[:, mc, kc, :],
                                     pooledT[:, kc, :, 0:NH],
                                     start=(kc == 0), stop=(kc == 5))
                for kc in range(6):
                    nc.tensor.matmul(ps_o, w1_sb[:, mc, 6 + kc, :],
                                     pooledT[:, kc, :, NH:D],
                                     start=(kc == 0), stop=(kc == 5))
                a_sb = tmpp.tile([128, BL, NH], BF16, tag="ab")
                nc.scalar.copy(a_sb, ps_h)
                for img in range(BL):
                    pre = tmpp.tile([128, NH, NO], BF16, tag="pre")
                    a_bc = a_sb[:, img, :][:, :, None].broadcast_to([128, NH, NO])
                    b_bc = ps_o[:, img, :][:, None, :].broadcast_to([128, NH, NO])
                    nc.vector.tensor_tensor(pre, a_bc, b_bc, op=add)
                    dst = x1T[:, mc, img * NPAIR:(img + 1) * NPAIR] \
                        .rearrange("p (i j) -> p i j", i=NH)
                    nc.scalar.activation(dst, pre, relu, bias=b1v[:, mc:mc + 1])

            # ---- layer 2 ----
            fillers(4)
            for m2 in range(2):
                ps2 = mps.tile([128, M], F32, tag="mm")
                for kc in range(4):
                    nc.tensor.matmul(ps2, w2v[:, kc, m2 * 128:(m2 + 1) * 128],
                                     x1T[:, kc, :], start=(kc == 0), stop=(kc == 3))
                nc.scalar.activation(x2T[:, m2, :], ps2, relu,
                                     bias=b2v[:, m2:m2 + 1])

            # ---- layer 3 + bias + single store ----
            fillers(3)
            o_sb = tmpp.tile([128, 3, H3], F32, tag="osb")
            for m3 in range(3):
                ps3 = mps.tile([128, H3], F32, tag="mm")
                for kc in range(2):
                    nc.tensor.matmul(ps3, x2T[:, kc, m3 * 128:(m3 + 1) * 128],
                                     w3v[:, kc, :], start=(kc == 0), stop=(kc == 1))
                nc.vector.tensor_tensor(o_sb[:, m3, :], ps3,
                                        b3v[:, 0:H3], op=add)
                nc.scalar.dma_start(out=out[:, m3, :], in_=o_sb[:, m3, :])
    nc.compile()
    return nc


def _get_program(nch):
    if nch not in _PROGRAMS:
        _PROGRAMS[nch] = _build_program(nch)
    return _PROGRAMS[nch]


def _preprocess(features, boxes, scores):
    """Pack union-of-boxes pixels; rasterize masks with detection columns in
    sorted-score order. Returns partition-major fp8 feat/mask + 1/area."""
    cx, cy, bw, bh = boxes[..., 0], boxes[..., 1], boxes[..., 2], boxes[..., 3]
    x1 = np.floor((cx - bw / 2) * GRID).astype(np.int64)
    y1 = np.floor((cy - bh / 2) * GRID).astype(np.int64)
    x2 = np.floor((cx + bw / 2) * GRID).astype(np.int64)
    y2 = np.floor((cy + bh / 2) * GRID).astype(np.int64)
    hidx = np.argsort(-scores[:, :NH], axis=1, kind="stable")
    oidx = np.argsort(-scores[:, NH:], axis=1, kind="stable") + NH
    perm = np.concatenate([hidx, oidx], axis=1)                     # [B, D]
    g = np.arange(GRID)
    rows = (g[None, None, :] >= y1[..., None]) & (g[None, None, :] < y2[..., None])
    cols = (g[None, None, :] >= x1[..., None]) & (g[None, None, :] < x2[..., None])
    rows = np.take_along_axis(rows, perm[..., None], axis=1)        # [B, D, 64]
    cols = np.take_along_axis(cols, perm[..., None], axis=1)
    area = rows.sum(-1) * cols.sum(-1)                              # [B, D]
    # union of all boxes per image; pack only covered pixels
    union = (rows[:, :, :, None] & cols[:, :, None, :]).any(axis=1)  # [B,64,64]
    uflat = union.reshape(B, GRID * GRID)
    counts = uflat.sum(1)
    nch = int(-(-counts.max() // 128))
    pp = nch * 128
    pix = np.zeros((B, pp), np.int64)
    valid = np.zeros((B, pp), bool)
    for b in range(B):
        idx = np.flatnonzero(uflat[b])
        pix[b, :len(idx)] = idx
        valid[b, :len(idx)] = True
    featP = features.reshape(B, GRID * GRID, C)[np.arange(B)[:, None], pix]
    featP[~valid] = 0.0
    py, px = pix // GRID, pix % GRID
    mrow = np.take_along_axis(rows, py[:, None, :], axis=2)          # [B, D, pp]
    mcol = np.take_along_axis(cols, px[:, None, :], axis=2)
    maskP = (mrow & mcol & valid[:, None, :]).transpose(0, 2, 1)     # [B, pp, D]
    return (featP.astype(F8), maskP.astype(F8),
            (1.0 / area).astype(np.float32), nch)


def _pmajor(a, nch):
    """[BL, nch*128, X] -> contiguous [128, BL, nch, X]."""
    bl, _, x = a.shape
    return np.ascontiguousarray(
        a.reshape(bl, nch, 128, x).transpose(2, 0, 1, 3))


def _run(in_maps, trace=False, **kw):
    nch = in_maps[0]["feat"].shape[2]
    nc = _get_program(nch)
    return run_bass_kernel_spmd(nc, in_maps, core_ids=list(range(N_CORES)),
                                trace=trace, **kw)


def _make_in_maps(features, boxes, scores, w1, b1, w2, b2, w3, b3):
    features = np.asarray(features, np.float32)
    featP, maskP, inva, nch = _preprocess(
        features, np.asarray(boxes, np.float32), np.asarray(scores, np.float32))
    # w1 [1536, 512] -> [128 part, 4 mc, 12 kc, 128]
    w1b = np.asarray(w1, np.float32).astype(BF).reshape(12, 128, 4, 128)
    w1b = np.ascontiguousarray(w1b.transpose(1, 2, 0, 3))
    wcb = np.zeros((128, WC_COLS), BF)
    wcb[:, W2_OFF:W2_OFF + 4 * H2] = np.asarray(w2, np.float32).astype(BF) \
        .reshape(4, 128, H2).transpose(1, 0, 2).reshape(128, 4 * H2)
    wcb[:, W3_OFF:W3_OFF + 2 * H3] = np.asarray(w3, np.float32).astype(BF) \
        .reshape(2, 128, H3).transpose(1, 0, 2).reshape(128, 2 * H3)
    wcb[:, B1_OFF:B1_OFF + 4] = np.asarray(b1, np.float32).astype(BF) \
        .reshape(4, 128).T
    wcb[:, B2_OFF:B2_OFF + 2] = np.asarray(b2, np.float32).astype(BF) \
        .reshape(2, 128).T
    wcb[:, B3_OFF:B3_OFF + H3] = np.asarray(b3, np.float32).astype(BF)[None, :]
    wcb = np.ascontiguousarray(wcb)
    in_maps = []
    for c in range(N_CORES):
        s = slice(c * BL, (c + 1) * BL)
        in_maps.append({
            "feat": _pmajor(featP[s], nch),
            "maskT": _pmajor(maskP[s], nch),
            "inva": np.ascontiguousarray(inva[s].T),
            "w1": w1b, "wc": wcb,
        })
    return in_maps


def kernel(features, boxes, scores, w1, b1, w2, b2, w3, b3, labels):
    in_maps = _make_in_maps(features, boxes, scores, w1, b1, w2, b2, w3, b3)
    res = _run(in_maps, trace=False)
    out = np.concatenate(
        [r["out"].transpose(1, 0, 2).reshape(BL, NPAIR, H3)
         for r in res.results], axis=0)
    return np.ascontiguousarray(out.astype(np.float32))
